# revision 1
# baseline (speedup 1.0000x reference)
"""Tensor-parallel GQA attention kernel for 8 Trainium2 NeuronCores.

Problem: x[2,2048,2048] -> Attention(16 q heads, 4 kv heads, rotary,
causal) -> out[2,2048,2048].

Sharding: core c handles batch b=c//4 and kv-group g=c%4 (4 q-heads +
1 kv-head). Each core computes its heads' attention output and a
partial O-projection [DIM, S] (output-dim major); the host sums the 4
partials per batch and transposes.

On-core dataflow (everything feature/dim-major so matmul contractions
land on the partition axis). All matmul operands are fp16 (fp32 PSUM
accumulation): fp16 gets fast-weight-load so LDWEIGHTS hides under the
previous matmul's stream, and runs 1 cycle/row at any free size.
  xT = transpose(x) via PE-transpose (fp16)
  QT/KT/VT = W.T @ xT
  RoPE applied per 512-chunk right after projection (overlaps PE work).
  Weight columns are pair-permuted on the host so partitions 0..63
  hold "real" dims, 64..127 "imag".
  scoresT[k,q] = KT_tile.T @ QT (pairs of k-tiles into one 2-bank
  PSUM tile) -> one exp per pair (ACT, ->fp16) -> mask (diag chunks)
  outT[dv,q] += V_tile.T @ attnT, sums[1,q] += ones.T @ attnT
  normalize via batched reciprocal_approx_fast + K=1 broadcast-matmul
  OT[o,q] += wo_tile.T @ outT
"""
import numpy as np

import concourse.bass as bass
import concourse.tile as tile
import concourse.mybir as mybir
from concourse import bacc
from concourse import bass_utils

F32 = mybir.dt.float32
F32R = mybir.dt.float32r
F16 = mybir.dt.float16

DIM = 2048
S = 2048
B = 2
HL = 4           # q heads per core
FT = DIM // 128  # feature tiles
TT = S // 128    # token tiles
CH = 2           # token chunks (1024 tokens each) for projections
QC = 4           # q chunks (512) for attention
SCALE = 1.0 / np.sqrt(128.0)

_CACHE = {}


def _build():
    nc = bacc.Bacc("TRN2", target_bir_lowering=False, debug=False,
                   enable_asserts=True, num_devices=8)

    d_x = nc.dram_tensor("x_c", (S, DIM), F16, kind="ExternalInput").ap()
    d_wq = nc.dram_tensor("wq_c", (DIM, HL * 128), F16, kind="ExternalInput").ap()
    d_wk = nc.dram_tensor("wk_c", (DIM, 128), F16, kind="ExternalInput").ap()
    d_wv = nc.dram_tensor("wv_c", (DIM, 128), F16, kind="ExternalInput").ap()
    d_wo = nc.dram_tensor("wo_c", (HL * 128, DIM), F16, kind="ExternalInput").ap()
    d_cj = nc.dram_tensor("cjoin", (128, S), F16, kind="ExternalInput").ap()
    d_sj = nc.dram_tensor("sjoin", (128, S), F16, kind="ExternalInput").ap()
    d_mk = nc.dram_tensor("masks", (4, 128, 512), F16, kind="ExternalInput").ap()
    d_id = nc.dram_tensor("ident", (128, 128), F16, kind="ExternalInput").ap()
    d_ot = nc.dram_tensor("ot", (DIM, S), F32, kind="ExternalOutput").ap()

    Exp = mybir.ActivationFunctionType.Exp

    with tile.TileContext(nc) as tc:
        with tc.tile_pool(name="wts", bufs=1) as wp, \
             tc.tile_pool(name="acts", bufs=1) as ap:
            sb_id = wp.tile([128, 128], F16)
            nc.sync.dma_start(sb_id[:], d_id)
            sb_wq = wp.tile([128, FT, HL * 128], F16)
            nc.sync.dma_start(sb_wq[:], d_wq.rearrange("(ft p) m -> p ft m", p=128))
            sb_wk = wp.tile([128, FT, 128], F16)
            nc.sync.dma_start(sb_wk[:], d_wk.rearrange("(ft p) m -> p ft m", p=128))
            sb_wv = wp.tile([128, FT, 128], F16)
            nc.sync.dma_start(sb_wv[:], d_wv.rearrange("(ft p) m -> p ft m", p=128))
            sb_cj = wp.tile([128, S], F16)
            sb_sj = wp.tile([128, S], F16)
            sb_mk = wp.tile([128, 4, 512], F16)
            sb_wo = wp.tile([128, HL, DIM], F16)
            ones16 = wp.tile([128, 1], F16)
            nc.vector.memset(ones16[:], 1.0)
            ones32 = wp.tile([1, 128], F32)
            nc.vector.memset(ones32[:], 1.0)

            sb_QT = ap.tile([128, HL, S], F16)
            sb_KT = ap.tile([128, S], F16)
            sb_V = ap.tile([128, TT, 128], F16)
            sb_oT = ap.tile([128, HL, S], F16)

            # ---- Phase A: DMA-transpose x + Q/K/V projections + RoPE per chunk
            with tc.tile_pool(name="xT", bufs=2) as xT_p, \
                 tc.tile_pool(name="vt", bufs=2) as vt_p, \
                 tc.tile_pool(name="rope", bufs=2) as rp, \
                 tc.tile_pool(name="ps_tr", bufs=2, space="PSUM") as ps_tr, \
                 tc.tile_pool(name="ps_pj", bufs=3, space="PSUM") as ps_pj:

                def rope(T, c0):
                    # T: [128, 512] fp16 chunk at token offset c0
                    mc = rp.tile([128, 512], F16, tag="mc")
                    ms = rp.tile([128, 512], F16, tag="ms")
                    cjs = sb_cj[:, c0:c0 + 512]
                    sjs = sb_sj[:, c0:c0 + 512]
                    nc.gpsimd.tensor_mul(mc[:], T, cjs)
                    nc.vector.tensor_mul(ms[0:64, :], T[64:128, :], sjs[64:128, :])
                    nc.vector.tensor_mul(ms[64:128, :], T[0:64, :], sjs[0:64, :])
                    nc.vector.tensor_add(T, mc[:], ms[:])

                for ch in range(CH):
                    c0 = ch * 1024
                    xt = xT_p.tile([128, FT, 1024], F16)
                    for fi in range(FT):
                        nc.sync.dma_start(
                            xt[:, fi, :],
                            d_x[c0:c0 + 1024, fi * 128:(fi + 1) * 128],
                            transpose=True)
                    if ch == 0:
                        nc.scalar.dma_start(sb_cj[:], d_cj)
                        nc.scalar.dma_start(sb_sj[:], d_sj)
                        nc.scalar.dma_start(sb_mk[:], d_mk.rearrange("m p n -> p m n"))
                        nc.scalar.dma_start(sb_wo[:], d_wo.rearrange("(dv p) m -> p dv m", p=128))
                    for sc_ in range(2):
                        s0 = c0 + sc_ * 512
                        xts = xt[:, :, sc_ * 512:(sc_ + 1) * 512]
                        for h in range(HL):
                            pq = ps_pj.tile([128, 512], F32, tag="pj")
                            for fi in range(FT):
                                nc.tensor.matmul(
                                    pq[:], sb_wq[:, fi, h * 128:(h + 1) * 128],
                                    xts[:, fi, :], start=(fi == 0),
                                    stop=(fi == FT - 1))
                            nc.vector.tensor_copy(sb_QT[:, h, s0:s0 + 512], pq[:])
                            rope(sb_QT[:, h, s0:s0 + 512], s0)
                        pk = ps_pj.tile([128, 512], F32, tag="pj")
                        for fi in range(FT):
                            nc.tensor.matmul(pk[:], sb_wk[:, fi, :], xts[:, fi, :],
                                             start=(fi == 0), stop=(fi == FT - 1))
                        nc.vector.tensor_copy(sb_KT[:, s0:s0 + 512], pk[:])
                        rope(sb_KT[:, s0:s0 + 512], s0)
                        pv = ps_pj.tile([128, 512], F32, tag="pj")
                        for fi in range(FT):
                            nc.tensor.matmul(pv[:], sb_wv[:, fi, :], xts[:, fi, :],
                                             start=(fi == 0), stop=(fi == FT - 1))
                        vt = vt_p.tile([128, 512], F16)
                        nc.vector.tensor_copy(vt[:], pv[:])
                        for tl in range(4):
                            ti = (ch * 2 + sc_) * 4 + tl
                            ptv = ps_tr.tile([128, 128], F16, tag="tr")
                            nc.tensor.transpose(
                                ptv[:], vt[:, tl * 128:(tl + 1) * 128], sb_id[:])
                            nc.vector.tensor_copy(sb_V[:, ti, :], ptv[:])

            # ---- Phase C: attention
            with tc.tile_pool(name="attn", bufs=4) as at_p, \
                 tc.tile_pool(name="bcst", bufs=2) as bc_p, \
                 tc.tile_pool(name="rcp", bufs=2) as rc_p, \
                 tc.tile_pool(name="ps_sc", bufs=2, space="PSUM") as ps_sc, \
                 tc.tile_pool(name="ps_o", bufs=2, space="PSUM") as ps_o, \
                 tc.tile_pool(name="ps_sum", bufs=1, space="PSUM") as ps_sum, \
                 tc.tile_pool(name="ps_bc", bufs=1, space="PSUM") as ps_bc:
                for qc in range(QC):
                    kmax = (qc + 1) * 4
                    q0 = qc * 512
                    for h in range(HL):
                        po = ps_o.tile([128, 512], F32, tag="po")
                        psum = ps_sum.tile([1, 512], F32, tag="ps")
                        for kp in range(kmax // 2):
                            psc = ps_sc.tile([128, 1024], F32, tag="sc")
                            at = at_p.tile([128, 1024], F16, tag="at")
                            for half in range(2):
                                ki = kp * 2 + half
                                nc.tensor.matmul(
                                    psc[:, half * 512:(half + 1) * 512],
                                    sb_KT[:, ki * 128:(ki + 1) * 128],
                                    sb_QT[:, h, q0:q0 + 512],
                                    start=True, stop=True)
                            nc.scalar.activation(at[:], psc[:], Exp, scale=SCALE)
                            for half in range(2):
                                ki = kp * 2 + half
                                if ki >= qc * 4:
                                    nc.vector.tensor_mul(
                                        at[:, half * 512:(half + 1) * 512],
                                        at[:, half * 512:(half + 1) * 512],
                                        sb_mk[:, ki - qc * 4, :])
                            for half in range(2):
                                ki = kp * 2 + half
                                nc.tensor.matmul(
                                    po[:], sb_V[:, ki, :],
                                    at[:, half * 512:(half + 1) * 512],
                                    start=(ki == 0), stop=(ki == kmax - 1))
                                nc.tensor.matmul(
                                    psum[:], ones16[:],
                                    at[:, half * 512:(half + 1) * 512],
                                    start=(ki == 0), stop=(ki == kmax - 1))
                        rc = rc_p.tile([1, 512], F32)
                        nc.vector.reciprocal_approx_fast(rc[:], psum[:])
                        pbc = ps_bc.tile([128, 512], F32, tag="bc")
                        nc.tensor.matmul(pbc[:], ones32[:], rc[:],
                                         start=True, stop=True)
                        bc = bc_p.tile([128, 512], F32)
                        nc.vector.tensor_copy(bc[:], pbc[:])
                        nc.vector.tensor_mul(
                            sb_oT[:, h, q0:q0 + 512], po[:], bc[:])

            # ---- Phase D: O projection
            with tc.tile_pool(name="otile", bufs=4) as ot_p, \
                 tc.tile_pool(name="ps_ot", bufs=4, space="PSUM") as ps_ot:
                for oi in range(FT):
                    for qc in range(QC):
                        pot = ps_ot.tile([128, 512], F32, tag="ot")
                        for dvi in range(HL):
                            nc.tensor.matmul(
                                pot[:], sb_wo[:, dvi, oi * 128:(oi + 1) * 128],
                                sb_oT[:, dvi, qc * 512:(qc + 1) * 512],
                                start=(dvi == 0), stop=(dvi == HL - 1))
                        otc = ot_p.tile([128, 512], F32)
                        if qc % 2 == 0:
                            nc.vector.tensor_copy(otc[:], pot[:])
                        else:
                            nc.scalar.copy(otc[:], pot[:])
                        nc.sync.dma_start(
                            d_ot[oi * 128:(oi + 1) * 128,
                                 qc * 512:(qc + 1) * 512], otc[:])

    nc.compile()
    return nc


def _prep_shards(x, freqs_cos, freqs_sin, wq, wk, wv, wo):
    perm = np.empty(128, dtype=np.int64)
    perm[0:64] = 2 * np.arange(64)
    perm[64:128] = 2 * np.arange(64) + 1

    cosT = np.ascontiguousarray(freqs_cos.T).astype(np.float32)
    sinT = np.ascontiguousarray(freqs_sin.T).astype(np.float32)
    cjoin = np.concatenate([cosT, cosT], axis=0).astype(np.float16)
    sjoin = np.concatenate([sinT, -sinT], axis=0).astype(np.float16)

    masks = np.zeros((4, 128, 512), dtype=np.float16)
    q_idx = np.arange(512)[None, :]
    k_idx = np.arange(128)[:, None]
    for m in range(4):
        masks[m] = (q_idx >= m * 128 + k_idx).astype(np.float16)
    ident = np.eye(128, dtype=np.float16)

    in_maps = []
    for c in range(8):
        b, g = c // 4, c % 4
        wq_g = np.ascontiguousarray(
            wq[:, g * 512:(g + 1) * 512].reshape(DIM, 4, 128)[:, :, perm]
            .reshape(DIM, 512)).astype(np.float16)
        wk_g = np.ascontiguousarray(
            wk[:, g * 128:(g + 1) * 128][:, perm]).astype(np.float16)
        wv_g = np.ascontiguousarray(
            wv[:, g * 128:(g + 1) * 128]).astype(np.float16)
        wo_g = np.ascontiguousarray(
            wo[g * 512:(g + 1) * 512, :]).astype(np.float16)
        in_maps.append({
            "x_c": np.ascontiguousarray(x[b]).astype(np.float16),
            "wq_c": wq_g, "wk_c": wk_g, "wv_c": wv_g, "wo_c": wo_g,
            "cjoin": cjoin, "sjoin": sjoin, "masks": masks, "ident": ident,
        })
    return in_maps


def _assemble(results):
    out = np.zeros((B, S, DIM), dtype=np.float32)
    for c in range(8):
        out[c // 4] += results[c]["ot"].T
    return out


def kernel(x, freqs_cos, freqs_sin, wq, wk, wv, wo):
    x = np.asarray(x, dtype=np.float32)
    if "nc" not in _CACHE:
        _CACHE["nc"] = _build()
    nc = _CACHE["nc"]
    in_maps = _prep_shards(x, np.asarray(freqs_cos), np.asarray(freqs_sin),
                           np.asarray(wq), np.asarray(wk), np.asarray(wv),
                           np.asarray(wo))
    res = bass_utils.run_bass_kernel_spmd(nc, in_maps, core_ids=list(range(8)))
    return _assemble(res.results)



# revision 4
# speedup vs baseline: 1.0178x; 1.0178x over previous
"""Tensor-parallel GQA attention kernel for 8 Trainium2 NeuronCores.

Problem: x[2,2048,2048] -> Attention(16 q heads, 4 kv heads, rotary,
causal) -> out[2,2048,2048].

Sharding: core c handles batch b=c//4 and kv-group g=c%4 (4 q-heads +
1 kv-head). Each core computes its heads' attention output and a
partial O-projection [DIM, S] (output-dim major, fp16); the host sums
the 4 partials per batch and transposes.

v2 design (pure fp16 matmuls, fp32 PSUM):
  - x is pre-transposed on the host (xT [DIM, S] fp16) so all loads are
    contiguous; chunk-0 load is split per feature-tile so the first
    projection matmul can start ~1us in. Dummy warmup matmuls run
    during the first DMA to lift the PE HAM throttle.
  - Projections QT/KT/VT = W.T @ xT per 512-token chunk; RoPE fused in
    (weight columns pair-permuted on host: partitions 0..63 real,
    64..127 imag).
  - Attention per 512-q chunk, per head, software-pipelined: the score
    matmuls for k-pair kp+1 are emitted before the AV matmuls of kp so
    the in-order PE queue never stalls on the exp (ACT) latency.
  - Causal trapezoid: diagonal k-tiles stream only the valid q suffix
    (N = 512/384/256/128); a single [128,128] triangle mask handles the
    diagonal blocks.
  - Softmax denominators: DVE accumulates exp partials per partition;
    gpsimd partition_all_reduce (add) reduces across partitions and
    broadcasts; DVE reciprocal + multiply normalizes. No PE ones-MMs.
  - O-projection for q-chunk qc-1 is interleaved between attention
    heads of chunk qc (keeps PE dense, spreads output DMA).
"""
import numpy as np

import concourse.bass as bass
import concourse.bass_isa as bass_isa
import concourse.tile as tile
import concourse.mybir as mybir
from concourse import bacc
from concourse import bass_utils

F32 = mybir.dt.float32
F16 = mybir.dt.float16

DIM = 2048
S = 2048
B = 2
HL = 4           # q heads per core
FT = DIM // 128  # feature tiles
QC = 4           # q chunks (512) for attention
SCALE = 1.0 / np.sqrt(128.0)

_CACHE = {}


def _build():
    nc = bacc.Bacc("TRN2", target_bir_lowering=False, debug=False,
                   enable_asserts=True, num_devices=8)

    d_xt = nc.dram_tensor("xt_c", (DIM, S), F16, kind="ExternalInput").ap()
    d_wq = nc.dram_tensor("wq_c", (DIM, HL * 128), F16, kind="ExternalInput").ap()
    d_wk = nc.dram_tensor("wk_c", (DIM, 128), F16, kind="ExternalInput").ap()
    d_wv = nc.dram_tensor("wv_c", (DIM, 128), F16, kind="ExternalInput").ap()
    d_wo = nc.dram_tensor("wo_c", (HL * 128, DIM), F16, kind="ExternalInput").ap()
    d_cj = nc.dram_tensor("cjoin", (128, S), F16, kind="ExternalInput").ap()
    d_sj = nc.dram_tensor("sjoin", (128, S), F16, kind="ExternalInput").ap()
    d_mk = nc.dram_tensor("maskt", (128, 128), F16, kind="ExternalInput").ap()
    d_id = nc.dram_tensor("ident", (128, 128), F16, kind="ExternalInput").ap()
    d_ot = nc.dram_tensor("ot", (DIM, S), F16, kind="ExternalOutput").ap()

    Exp = mybir.ActivationFunctionType.Exp
    v_xt = d_xt.rearrange("(ft p) s -> p ft s", p=128)

    with tile.TileContext(nc) as tc:
        with tc.tile_pool(name="wts", bufs=1) as wp, \
             tc.tile_pool(name="acts", bufs=1) as ap:
            sb_id = wp.tile([128, 128], F16)
            nc.sync.dma_start(sb_id[:], d_id)
            sb_wq = wp.tile([128, FT, HL * 128], F16)
            v_wq = d_wq.rearrange("(ft p) m -> p ft m", p=128)
            for fi in range(FT):
                nc.scalar.dma_start(sb_wq[:, fi, :], v_wq[:, fi, :])
            sb_wk = wp.tile([128, FT, 128], F16)
            nc.scalar.dma_start(sb_wk[:], d_wk.rearrange("(ft p) m -> p ft m", p=128))
            sb_wv = wp.tile([128, FT, 128], F16)
            nc.scalar.dma_start(sb_wv[:], d_wv.rearrange("(ft p) m -> p ft m", p=128))
            sb_cj = wp.tile([128, S], F16)
            sb_sj = wp.tile([128, S], F16)
            sb_mk = wp.tile([128, 128], F16)
            sb_wo = wp.tile([128, HL, DIM], F16)

            sb_QT = ap.tile([128, HL, S], F16)
            sb_KT = ap.tile([128, S], F16)
            sb_V = ap.tile([128, S // 128, 128], F16)
            sb_oT = ap.tile([128, HL, S], F16)

            # ---- Phase A: Q/K/V projections + RoPE per 512-token chunk
            with tc.tile_pool(name="xT", bufs=2) as xT_p, \
                 tc.tile_pool(name="vt", bufs=2) as vt_p, \
                 tc.tile_pool(name="rope", bufs=2) as rp, \
                 tc.tile_pool(name="ps_wm", bufs=1, space="PSUM") as ps_wm, \
                 tc.tile_pool(name="ps_tr", bufs=2, space="PSUM") as ps_tr, \
                 tc.tile_pool(name="ps_pj", bufs=3, space="PSUM") as ps_pj:

                # PE warmup: lift the HAM clock gate while chunk-0 DMA lands
                pwarm = ps_wm.tile([128, 128], F32, tag="wm")
                for _ in range(40):
                    nc.tensor.matmul(pwarm[:], sb_id[:], sb_id[:],
                                     start=True, stop=True)

                def rope(T, c0):
                    # T: [128, 512] fp16 chunk at token offset c0
                    mc = rp.tile([128, 512], F16, tag="mc")
                    ms = rp.tile([128, 512], F16, tag="ms")
                    cjs = sb_cj[:, c0:c0 + 512]
                    sjs = sb_sj[:, c0:c0 + 512]
                    nc.gpsimd.tensor_mul(mc[:], T, cjs)
                    nc.vector.tensor_mul(ms[0:64, :], T[64:128, :], sjs[64:128, :])
                    nc.vector.tensor_mul(ms[64:128, :], T[0:64, :], sjs[0:64, :])
                    nc.vector.tensor_add(T, mc[:], ms[:])

                for sc in range(4):
                    s0 = sc * 512
                    xt = xT_p.tile([128, FT, 512], F16)
                    if sc == 0:
                        nc.gpsimd.dma_start(sb_cj[:], d_cj)
                        nc.gpsimd.dma_start(sb_sj[:], d_sj)
                        for fi in range(FT):
                            eng = nc.sync if fi % 2 == 0 else nc.gpsimd
                            eng.dma_start(xt[:, fi, :], v_xt[:, fi, s0:s0 + 512])
                        nc.gpsimd.dma_start(sb_mk[:], d_mk)
                        nc.gpsimd.dma_start(
                            sb_wo[:], d_wo.rearrange("(dv p) m -> p dv m", p=128))
                    else:
                        nc.sync.dma_start(xt[:], v_xt[:, :, s0:s0 + 512])
                    for h in range(HL):
                        pq = ps_pj.tile([128, 512], F32, tag="pj")
                        for fi in range(FT):
                            nc.tensor.matmul(
                                pq[:], sb_wq[:, fi, h * 128:(h + 1) * 128],
                                xt[:, fi, :], start=(fi == 0),
                                stop=(fi == FT - 1))
                        nc.scalar.copy(sb_QT[:, h, s0:s0 + 512], pq[:])
                        rope(sb_QT[:, h, s0:s0 + 512], s0)
                    pk = ps_pj.tile([128, 512], F32, tag="pj")
                    for fi in range(FT):
                        nc.tensor.matmul(pk[:], sb_wk[:, fi, :], xt[:, fi, :],
                                         start=(fi == 0), stop=(fi == FT - 1))
                    nc.scalar.copy(sb_KT[:, s0:s0 + 512], pk[:])
                    rope(sb_KT[:, s0:s0 + 512], s0)
                    pv = ps_pj.tile([128, 512], F32, tag="pj")
                    for fi in range(FT):
                        nc.tensor.matmul(pv[:], sb_wv[:, fi, :], xt[:, fi, :],
                                         start=(fi == 0), stop=(fi == FT - 1))
                    vt = vt_p.tile([128, 512], F16)
                    nc.scalar.copy(vt[:], pv[:])
                    for tl in range(4):
                        ti = sc * 4 + tl
                        ptv = ps_tr.tile([128, 128], F16, tag="tr")
                        nc.tensor.transpose(
                            ptv[:], vt[:, tl * 128:(tl + 1) * 128], sb_id[:])
                        nc.vector.tensor_copy(sb_V[:, ti, :], ptv[:])

            # ---- Phase C/D: attention with interleaved O-projection
            with tc.tile_pool(name="attn", bufs=4) as at_p, \
                 tc.tile_pool(name="acc", bufs=2) as ac_p, \
                 tc.tile_pool(name="dnm", bufs=2) as dn_p, \
                 tc.tile_pool(name="rcf", bufs=2) as rc_p, \
                 tc.tile_pool(name="otile", bufs=4) as ot_p, \
                 tc.tile_pool(name="ps_sc", bufs=2, space="PSUM") as ps_sc, \
                 tc.tile_pool(name="ps_o", bufs=2, space="PSUM") as ps_o, \
                 tc.tile_pool(name="ps_ot", bufs=2, space="PSUM") as ps_ot:

                def oproj_group(qc, oi, engine):
                    # out[:, qc block] partial for output tile oi
                    pot = ps_ot.tile([128, 512], F32, tag="ot")
                    for dvi in range(HL):
                        nc.tensor.matmul(
                            pot[:], sb_wo[:, dvi, oi * 128:(oi + 1) * 128],
                            sb_oT[:, dvi, qc * 512:(qc + 1) * 512],
                            start=(dvi == 0), stop=(dvi == HL - 1))
                    otc = ot_p.tile([128, 512], F16)
                    if engine == "v":
                        nc.vector.tensor_copy(otc[:], pot[:])
                    else:
                        nc.scalar.copy(otc[:], pot[:])
                    nc.sync.dma_start(
                        d_ot[oi * 128:(oi + 1) * 128,
                             qc * 512:(qc + 1) * 512], otc[:])

                def attn_head(qc, h):
                    kmax = (qc + 1) * 4
                    q0 = qc * 512
                    po = ps_o.tile([128, 512], F32, tag="po")
                    accum = ac_p.tile([128, 512], F16)

                    def width(ki):
                        # valid q-suffix width for k-tile ki in this chunk
                        jloc = ki - qc * 4
                        return 512 if jloc < 0 else 512 - jloc * 128

                    def emit_scores(kp):
                        psc = ps_sc.tile([128, 1024], F32, tag="sc")
                        at = at_p.tile([128, 1024], F16, tag="at")
                        for half in range(2):
                            ki = kp * 2 + half
                            w = width(ki)
                            qo = 512 - w
                            nc.tensor.matmul(
                                psc[:, half * 512 + qo:(half + 1) * 512],
                                sb_KT[:, ki * 128:(ki + 1) * 128],
                                sb_QT[:, h, q0 + qo:q0 + 512],
                                start=True, stop=True)
                        w0, w1 = width(kp * 2), width(kp * 2 + 1)
                        if w0 == 512 and w1 == 512:
                            nc.scalar.activation(at[:], psc[:], Exp, scale=SCALE)
                        else:
                            nc.scalar.activation(
                                at[:, 512 - w0:512], psc[:, 512 - w0:512],
                                Exp, scale=SCALE)
                            nc.scalar.activation(
                                at[:, 1024 - w1:1024], psc[:, 1024 - w1:1024],
                                Exp, scale=SCALE)
                        # triangle mask on diagonal blocks, then denominator
                        # partial accumulation (DVE)
                        for half in range(2):
                            ki = kp * 2 + half
                            w = width(ki)
                            qo = 512 - w
                            c0 = half * 512 + qo
                            if ki >= qc * 4:
                                nc.vector.tensor_mul(
                                    at[:, c0:c0 + 128], at[:, c0:c0 + 128],
                                    sb_mk[:])
                            if ki == 0:
                                nc.vector.tensor_copy(accum[:], at[:, 0:512])
                            else:
                                nc.vector.tensor_add(
                                    accum[:, qo:512], accum[:, qo:512],
                                    at[:, c0:half * 512 + 512])
                        return at

                    def emit_av(kp, at):
                        for half in range(2):
                            ki = kp * 2 + half
                            w = width(ki)
                            qo = 512 - w
                            nc.tensor.matmul(
                                po[:, qo:512] if qo else po[:],
                                sb_V[:, ki, :],
                                at[:, half * 512 + qo:half * 512 + 512],
                                start=(ki == 0), stop=(ki == kmax - 1))

                    prev = None
                    for kp in range(kmax // 2):
                        at = emit_scores(kp)
                        if prev is not None:
                            emit_av(*prev)
                        prev = (kp, at)
                    emit_av(*prev)

                    dnm = dn_p.tile([128, 512], F32)
                    nc.gpsimd.partition_all_reduce(
                        dnm[:], accum[:], 128, bass_isa.ReduceOp.add)
                    rcf = rc_p.tile([128, 512], F32)
                    nc.vector.reciprocal_approx_fast(rcf[:], dnm[:])
                    nc.vector.tensor_mul(
                        sb_oT[:, h, q0:q0 + 512], po[:], rcf[:])

                for qc in range(QC):
                    for h in range(HL):
                        attn_head(qc, h)
                        if qc > 0:
                            # O-proj of the previous q-chunk, 4 tiles per head
                            eng = "s" if qc == 1 else "v"
                            for oi in range(h * 4, h * 4 + 4):
                                oproj_group(qc - 1, oi, eng)
                # tail: O-proj of the last q-chunk
                for oi in range(FT):
                    oproj_group(QC - 1, oi, "v" if oi % 2 else "s")

    nc.compile()
    return nc


def _prep_shards(x, freqs_cos, freqs_sin, wq, wk, wv, wo):
    perm = np.empty(128, dtype=np.int64)
    perm[0:64] = 2 * np.arange(64)
    perm[64:128] = 2 * np.arange(64) + 1

    cosT = np.ascontiguousarray(freqs_cos.T).astype(np.float32)
    sinT = np.ascontiguousarray(freqs_sin.T).astype(np.float32)
    cjoin = np.concatenate([cosT, cosT], axis=0).astype(np.float16)
    sjoin = np.concatenate([sinT, -sinT], axis=0).astype(np.float16)

    # triangle mask for diagonal 128x128 blocks: valid iff q >= k
    q_idx = np.arange(128)[None, :]
    k_idx = np.arange(128)[:, None]
    maskt = (q_idx >= k_idx).astype(np.float16)
    ident = np.eye(128, dtype=np.float16)

    xT = [np.ascontiguousarray(np.asarray(x[b]).T).astype(np.float16)
          for b in range(B)]

    in_maps = []
    for c in range(8):
        b, g = c // 4, c % 4
        wq_g = np.ascontiguousarray(
            wq[:, g * 512:(g + 1) * 512].reshape(DIM, 4, 128)[:, :, perm]
            .reshape(DIM, 512)).astype(np.float16)
        wk_g = np.ascontiguousarray(
            wk[:, g * 128:(g + 1) * 128][:, perm]).astype(np.float16)
        wv_g = np.ascontiguousarray(
            wv[:, g * 128:(g + 1) * 128]).astype(np.float16)
        wo_g = np.ascontiguousarray(
            wo[g * 512:(g + 1) * 512, :]).astype(np.float16)
        in_maps.append({
            "xt_c": xT[b],
            "wq_c": wq_g, "wk_c": wk_g, "wv_c": wv_g, "wo_c": wo_g,
            "cjoin": cjoin, "sjoin": sjoin, "maskt": maskt, "ident": ident,
        })
    return in_maps


def _assemble(results):
    out = np.zeros((B, S, DIM), dtype=np.float32)
    for c in range(8):
        out[c // 4] += results[c]["ot"].T.astype(np.float32)
    return out


def kernel(x, freqs_cos, freqs_sin, wq, wk, wv, wo):
    x = np.asarray(x, dtype=np.float32)
    if "nc" not in _CACHE:
        _CACHE["nc"] = _build()
    nc = _CACHE["nc"]
    in_maps = _prep_shards(x, np.asarray(freqs_cos), np.asarray(freqs_sin),
                           np.asarray(wq), np.asarray(wk), np.asarray(wv),
                           np.asarray(wo))
    res = bass_utils.run_bass_kernel_spmd(nc, in_maps, core_ids=list(range(8)))
    return _assemble(res.results)


# revision 8
# speedup vs baseline: 1.1407x; 1.1208x over previous
"""Tensor-parallel GQA attention kernel for 8 Trainium2 NeuronCores.

Problem: x[2,2048,2048] -> Attention(16 q heads, 4 kv heads, rotary,
causal) -> out[2,2048,2048].

Sharding: core c handles batch b=c//4 and kv-group g=c%4 (4 q-heads +
1 kv-head). Each core computes its heads' attention output and a
partial O-projection [DIM, S] (output-dim major, fp16); the host sums
the 4 partials per batch and transposes.

v3 design (pure fp16 matmuls, fp32 PSUM):
  - x pre-transposed on host (xT [DIM,S] fp16); all DMA loads are
    contiguous, chunk-0 split per feature-tile and spread over the
    sync/gpsimd queues in consumption order so the projection matmuls
    start ~1us after the DMA rings wake up. Warmup matmuls on a
    memset tile lift the HAM clock gate during DMA startup.
  - Projections run fi-outer with 6 concurrently-open PSUM
    accumulation groups (v,k,q0..q3) per 512-token chunk; evacuations
    on ScalarE, RoPE on DVE+GpSimd, V transposed via PE.
  - Attention per 512-q chunk per head, software-pipelined emission:
    score matmuls for k-pair kp+1 are issued before the AV/ones
    matmuls of pair kp so the in-order PE queue hides the exp (ACT)
    latency. Causal trapezoid: diagonal k-tiles stream only the valid
    q suffix (512/384/256/128); one [128,128] triangle mask.
  - Softmax denominators: ones-matmul [1,512] PSUM accumulation per
    k-tile (PE), reciprocal on DVE, partition-broadcast via a K=1
    matmul into the O-proj PSUM ring.
  - O-projection of q-chunk qc-1 interleaved between attention heads
    of chunk qc; fp16 partial outputs, host sums in fp32.
"""
import numpy as np

import concourse.bass as bass
import concourse.tile as tile
import concourse.mybir as mybir
from concourse import bacc
from concourse import bass_utils

F32 = mybir.dt.float32
F16 = mybir.dt.float16

DIM = 2048
S = 2048
B = 2
HL = 4           # q heads per core
FT = DIM // 128  # feature tiles
QC = 4           # q chunks (512) for attention
SCALE = 1.0 / np.sqrt(128.0)

_CACHE = {}


def _build():
    nc = bacc.Bacc("TRN2", target_bir_lowering=False, debug=False,
                   enable_asserts=True, num_devices=8)

    d_xt = nc.dram_tensor("xt_c", (DIM, S), F16, kind="ExternalInput").ap()
    d_wq = nc.dram_tensor("wq_c", (DIM, HL * 128), F16, kind="ExternalInput").ap()
    d_wk = nc.dram_tensor("wk_c", (DIM, 128), F16, kind="ExternalInput").ap()
    d_wv = nc.dram_tensor("wv_c", (DIM, 128), F16, kind="ExternalInput").ap()
    d_wo = nc.dram_tensor("wo_c", (HL * 128, DIM), F16, kind="ExternalInput").ap()
    d_cj = nc.dram_tensor("cjoin", (128, S), F16, kind="ExternalInput").ap()
    d_sj = nc.dram_tensor("sjoin", (128, S), F16, kind="ExternalInput").ap()
    d_mk = nc.dram_tensor("maskt", (128, 128), F16, kind="ExternalInput").ap()
    d_id = nc.dram_tensor("ident", (128, 128), F16, kind="ExternalInput").ap()
    d_ot = nc.dram_tensor("ot", (DIM, S), F16, kind="ExternalOutput").ap()

    Exp = mybir.ActivationFunctionType.Exp
    v_xt = d_xt.rearrange("(ft p) s -> p ft s", p=128)
    v_wq = d_wq.rearrange("(ft p) m -> p ft m", p=128)

    with tile.TileContext(nc) as tc:
        with tc.tile_pool(name="wts", bufs=1) as wp, \
             tc.tile_pool(name="acts", bufs=1) as ap:
            sb_warm = wp.tile([128, 128], F16)
            nc.vector.memset(sb_warm[:], 0.5)
            sb_id = wp.tile([128, 128], F16)
            sb_wq = wp.tile([128, FT, HL * 128], F16)
            sb_wk = wp.tile([128, FT, 128], F16)
            sb_wv = wp.tile([128, FT, 128], F16)
            sb_cj = wp.tile([128, S], F16)
            sb_sj = wp.tile([128, S], F16)
            sb_mk = wp.tile([128, 128], F16)
            sb_wo = wp.tile([128, HL, DIM], F16)
            ones16 = wp.tile([128, 1], F16)
            nc.vector.memset(ones16[:], 1.0)
            ones32 = wp.tile([1, 128], F32)
            nc.vector.memset(ones32[:], 1.0)

            sb_QT = ap.tile([128, HL, S], F16)
            sb_KT = ap.tile([128, S], F16)
            sb_V = ap.tile([128, S // 128, 128], F16)
            sb_oT = ap.tile([128, HL, S], F16)

            # ---- Phase A: Q/K/V projections + RoPE per 512-token chunk
            with tc.tile_pool(name="xT", bufs=2) as xT_p, \
                 tc.tile_pool(name="vt", bufs=2) as vt_p, \
                 tc.tile_pool(name="rope", bufs=2) as rp, \
                 tc.tile_pool(name="ps_tr", bufs=2, space="PSUM") as ps_tr, \
                 tc.tile_pool(name="ps_pj", bufs=1, space="PSUM") as ps_pj:

                # PE warmup on a memset tile: lifts the HAM clock gate
                # while the DMA rings start up (~9us); no DMA dependency.
                # Shares the "v" PSUM ring (released before the first
                # real V-projection matmul).
                pwarm = ps_pj.tile([128, 512], F32, tag="v")
                for _ in range(60):
                    nc.tensor.matmul(pwarm[:, 0:128], sb_warm[:], sb_warm[:],
                                     start=True, stop=True)

                def rope(T, c0):
                    # T: [128, 512] fp16 chunk at token offset c0
                    mc = rp.tile([128, 512], F16, tag="mc")
                    ms = rp.tile([128, 512], F16, tag="ms")
                    cjs = sb_cj[:, c0:c0 + 512]
                    sjs = sb_sj[:, c0:c0 + 512]
                    nc.gpsimd.tensor_mul(mc[:], T, cjs)
                    nc.vector.tensor_mul(ms[0:64, :], T[64:128, :], sjs[64:128, :])
                    nc.vector.tensor_mul(ms[64:128, :], T[0:64, :], sjs[0:64, :])
                    nc.vector.tensor_add(T, mc[:], ms[:])

                for sc in range(4):
                    s0 = sc * 512
                    xt = xT_p.tile([128, FT, 512], F16)
                    if sc == 0:
                        # DMA layout in consumption order: wk/wv first
                        # (v,k matmuls lead each fi group), xt per-fi
                        # split across two queues, wq per-fi on scalar.
                        nc.gpsimd.dma_start(sb_wk[:], d_wk.rearrange(
                            "(ft p) m -> p ft m", p=128))
                        nc.gpsimd.dma_start(sb_wv[:], d_wv.rearrange(
                            "(ft p) m -> p ft m", p=128))
                        for fi in range(FT):
                            nc.scalar.dma_start(sb_wq[:, fi, :], v_wq[:, fi, :])
                            eng = nc.sync if fi % 2 == 0 else nc.gpsimd
                            eng.dma_start(xt[:, fi, :], v_xt[:, fi, s0:s0 + 512])
                        nc.sync.dma_start(sb_id[:], d_id)
                        nc.gpsimd.dma_start(sb_cj[:], d_cj)
                        nc.gpsimd.dma_start(sb_sj[:], d_sj)
                        nc.gpsimd.dma_start(sb_mk[:], d_mk)
                        nc.gpsimd.dma_start(
                            sb_wo[:], d_wo.rearrange("(dv p) m -> p dv m", p=128))
                    else:
                        nc.sync.dma_start(xt[:], v_xt[:, :, s0:s0 + 512])

                    pv = ps_pj.tile([128, 512], F32, tag="v")
                    pk = ps_pj.tile([128, 512], F32, tag="k")
                    pq = [ps_pj.tile([128, 512], F32, tag=f"q{h}",
                                     name=f"pq{h}")
                          for h in range(HL)]
                    for fi in range(FT):
                        st, sp = (fi == 0), (fi == FT - 1)
                        nc.tensor.matmul(pv[:], sb_wv[:, fi, :], xt[:, fi, :],
                                         start=st, stop=sp)
                        nc.tensor.matmul(pk[:], sb_wk[:, fi, :], xt[:, fi, :],
                                         start=st, stop=sp)
                        for h in range(HL):
                            nc.tensor.matmul(
                                pq[h][:], sb_wq[:, fi, h * 128:(h + 1) * 128],
                                xt[:, fi, :], start=st, stop=sp)
                    vt = vt_p.tile([128, 512], F16)
                    nc.scalar.copy(vt[:], pv[:])
                    nc.scalar.copy(sb_KT[:, s0:s0 + 512], pk[:])
                    for h in range(HL):
                        nc.scalar.copy(sb_QT[:, h, s0:s0 + 512], pq[h][:])
                    for tl in range(4):
                        ptv = ps_tr.tile([128, 128], F16, tag="tr")
                        nc.tensor.transpose(
                            ptv[:], vt[:, tl * 128:(tl + 1) * 128], sb_id[:])
                        nc.vector.tensor_copy(sb_V[:, sc * 4 + tl, :], ptv[:])
                    rope(sb_KT[:, s0:s0 + 512], s0)
                    for h in range(HL):
                        rope(sb_QT[:, h, s0:s0 + 512], s0)

            # ---- Phase C/D: attention with interleaved O-projection
            with tc.tile_pool(name="attn", bufs=4) as at_p, \
                 tc.tile_pool(name="rcp", bufs=2) as rc_p, \
                 tc.tile_pool(name="bcst", bufs=2) as bc_p, \
                 tc.tile_pool(name="otile", bufs=4) as ot_p, \
                 tc.tile_pool(name="ps_sc", bufs=2, space="PSUM") as ps_sc, \
                 tc.tile_pool(name="ps_o", bufs=1, space="PSUM") as ps_o, \
                 tc.tile_pool(name="ps_sum", bufs=1, space="PSUM") as ps_sum, \
                 tc.tile_pool(name="ps_ot", bufs=2, space="PSUM") as ps_ot:

                def oproj_group(qc, oi, engine):
                    pot = ps_ot.tile([128, 512], F32, tag="ot")
                    for dvi in range(HL):
                        nc.tensor.matmul(
                            pot[:], sb_wo[:, dvi, oi * 128:(oi + 1) * 128],
                            sb_oT[:, dvi, qc * 512:(qc + 1) * 512],
                            start=(dvi == 0), stop=(dvi == HL - 1))
                    otc = ot_p.tile([128, 512], F16)
                    if engine == "v":
                        nc.vector.tensor_copy(otc[:], pot[:])
                    else:
                        nc.scalar.copy(otc[:], pot[:])
                    nc.sync.dma_start(
                        d_ot[oi * 128:(oi + 1) * 128,
                             qc * 512:(qc + 1) * 512], otc[:])

                def attn_head(qc, h):
                    kmax = (qc + 1) * 4
                    q0 = qc * 512
                    po = ps_o.tile([128, 512], F32, tag="po")
                    psum = ps_sum.tile([1, 512], F32, tag="ps")

                    def width(ki):
                        jloc = ki - qc * 4
                        return 512 if jloc < 0 else 512 - jloc * 128

                    def emit_scores(kp):
                        psc = ps_sc.tile([128, 1024], F32, tag="sc")
                        at = at_p.tile([128, 1024], F16, tag="at")
                        for half in range(2):
                            ki = kp * 2 + half
                            qo = 512 - width(ki)
                            nc.tensor.matmul(
                                psc[:, half * 512 + qo:(half + 1) * 512],
                                sb_KT[:, ki * 128:(ki + 1) * 128],
                                sb_QT[:, h, q0 + qo:q0 + 512],
                                start=True, stop=True)
                        w0, w1 = width(kp * 2), width(kp * 2 + 1)
                        if w0 == 512 and w1 == 512:
                            nc.scalar.activation(at[:], psc[:], Exp, scale=SCALE)
                        else:
                            nc.scalar.activation(
                                at[:, 512 - w0:512], psc[:, 512 - w0:512],
                                Exp, scale=SCALE)
                            nc.scalar.activation(
                                at[:, 1024 - w1:1024], psc[:, 1024 - w1:1024],
                                Exp, scale=SCALE)
                        for half in range(2):
                            ki = kp * 2 + half
                            if ki >= qc * 4:
                                c0 = half * 512 + 512 - width(ki)
                                nc.vector.tensor_mul(
                                    at[:, c0:c0 + 128], at[:, c0:c0 + 128],
                                    sb_mk[:])
                        return at

                    def emit_av(kp, at):
                        for half in range(2):
                            ki = kp * 2 + half
                            qo = 512 - width(ki)
                            st, sp = (ki == 0), (ki == kmax - 1)
                            rhs = at[:, half * 512 + qo:half * 512 + 512]
                            nc.tensor.matmul(
                                po[:, qo:512] if qo else po[:],
                                sb_V[:, ki, :], rhs, start=st, stop=sp)
                            nc.tensor.matmul(
                                psum[:, qo:512] if qo else psum[:],
                                ones16[:], rhs, start=st, stop=sp)

                    prev = None
                    for kp in range(kmax // 2):
                        at = emit_scores(kp)
                        if prev is not None:
                            emit_av(*prev)
                        prev = (kp, at)
                    emit_av(*prev)

                    rc = rc_p.tile([1, 512], F32)
                    nc.vector.reciprocal_approx_fast(rc[:], psum[:])
                    pbc = ps_ot.tile([128, 512], F32, tag="ot")
                    nc.tensor.matmul(pbc[:], ones32[:], rc[:],
                                     start=True, stop=True)
                    bc = bc_p.tile([128, 512], F32)
                    nc.vector.tensor_copy(bc[:], pbc[:])
                    nc.vector.tensor_mul(
                        sb_oT[:, h, q0:q0 + 512], po[:], bc[:])

                for qc in range(QC):
                    for h in range(HL):
                        attn_head(qc, h)
                        if qc > 0:
                            eng = "s" if qc == 1 else "v"
                            for oi in range(h * 4, h * 4 + 4):
                                oproj_group(qc - 1, oi, eng)
                for oi in range(FT):
                    oproj_group(QC - 1, oi, "v" if oi % 2 else "s")

    nc.compile()
    return nc


def _prep_shards(x, freqs_cos, freqs_sin, wq, wk, wv, wo):
    perm = np.empty(128, dtype=np.int64)
    perm[0:64] = 2 * np.arange(64)
    perm[64:128] = 2 * np.arange(64) + 1

    cosT = np.ascontiguousarray(freqs_cos.T).astype(np.float32)
    sinT = np.ascontiguousarray(freqs_sin.T).astype(np.float32)
    cjoin = np.concatenate([cosT, cosT], axis=0).astype(np.float16)
    sjoin = np.concatenate([sinT, -sinT], axis=0).astype(np.float16)

    q_idx = np.arange(128)[None, :]
    k_idx = np.arange(128)[:, None]
    maskt = (q_idx >= k_idx).astype(np.float16)
    ident = np.eye(128, dtype=np.float16)

    xT = [np.ascontiguousarray(np.asarray(x[b]).T).astype(np.float16)
          for b in range(B)]

    in_maps = []
    for c in range(8):
        b, g = c // 4, c % 4
        wq_g = np.ascontiguousarray(
            wq[:, g * 512:(g + 1) * 512].reshape(DIM, 4, 128)[:, :, perm]
            .reshape(DIM, 512)).astype(np.float16)
        wk_g = np.ascontiguousarray(
            wk[:, g * 128:(g + 1) * 128][:, perm]).astype(np.float16)
        wv_g = np.ascontiguousarray(
            wv[:, g * 128:(g + 1) * 128]).astype(np.float16)
        wo_g = np.ascontiguousarray(
            wo[g * 512:(g + 1) * 512, :]).astype(np.float16)
        in_maps.append({
            "xt_c": xT[b],
            "wq_c": wq_g, "wk_c": wk_g, "wv_c": wv_g, "wo_c": wo_g,
            "cjoin": cjoin, "sjoin": sjoin, "maskt": maskt, "ident": ident,
        })
    return in_maps


def _assemble(results):
    out = np.zeros((B, S, DIM), dtype=np.float32)
    for c in range(8):
        out[c // 4] += results[c]["ot"].T.astype(np.float32)
    return out


def kernel(x, freqs_cos, freqs_sin, wq, wk, wv, wo):
    x = np.asarray(x, dtype=np.float32)
    if "nc" not in _CACHE:
        _CACHE["nc"] = _build()
    nc = _CACHE["nc"]
    in_maps = _prep_shards(x, np.asarray(freqs_cos), np.asarray(freqs_sin),
                           np.asarray(wq), np.asarray(wk), np.asarray(wv),
                           np.asarray(wo))
    res = bass_utils.run_bass_kernel_spmd(nc, in_maps, core_ids=list(range(8)))
    return _assemble(res.results)
